# revision 1
# baseline (speedup 1.0000x reference)
"""MoE (dense routing) Trainium2 kernel: 8-core data-parallel over tokens.

Problem: nn_MixtureOfExperts_33011118637071
  N=16384 tokens, D=256 model dim, E=8 experts, H=128 gate hidden.
  gate   = softmax(relu(x @ Wg1 + bg1) @ Wg2 + bg2)          [N, E]
  h_e    = relu(x @ W1[e] + b1[e])                           [N, D]
  y      = sum_e gate[:, e] * (h_e @ W2[e] + b2[e])          [N, D]

Strategy (per core, 2048 tokens):
  Feature-major layout (features on partitions, tokens on the free dim) so
  the two expert GEMMs chain without transposes; x is transposed on the
  host as part of sharding and the output transposed back on gather.
  Matmuls run in float32r (full PE rate, ~tf32 accuracy).

  v2 over the original baseline (77.0us -> ~52us marginal):
  - All K<=8 one-hot/broadcast matmuls (gate-row broadcast, b2 init,
    exp-sum, 1/sum broadcast) are packed 3-per-pass into disjoint 32-row
    strips of the PE array via tile_position so they overlap instead of
    each paying a full N=512 streaming pass. The gate logits are computed
    replicated 4x across partition strips (Wg2 replicated host-side) so
    every strip has the gate rows it needs.
  - The softmax normalization is applied once per token tile to the
    replicated exp rows (one DVE multiply against the PE-broadcast 1/sum),
    so expert outputs accumulate already-normalized in PSUM and the final
    evacuation is a ScalarE copy instead of a VectorE multiply - removing
    the end-of-tile DVE dependency that stalled the next tile's PSUM bank
    reuse.
  - exp-sum matmuls interleave into the gate phase; 1/sum broadcasts are
    software-pipelined one tile ahead so no PE stall waits on them.
  - All small constants (gate weights, one-hot masks, b1, b2 blocks) are
    fused into one [128, 1298] host-built tensor loaded with a single DMA.
  - PSUM: quad pool (3 banks) for gate/broadcast passes + 2 output-accum
    banks + 3 hidden-layer banks = 8. The 3 hidden banks (vs 2) give the
    ScalarE relu chain slack so the PE never waits on a phid bank.

  Measured pitfalls kept out of this design: putting the PSUM->SBUF output
  evacuation (or a fused per-expert gate multiply) on the VectorE queue
  regressed ~10-40us/iter - the in-order DVE queue head-of-line blocks the
  next tile's short-dependency multiplies behind a long tile-tail
  dependency. Both output copies stay on ScalarE.
"""
import numpy as np

import bass_rust
import concourse.bass as bass
import concourse.mybir as mybir
import concourse.tile as tile
from concourse.bass_utils import run_bass_kernel_spmd

F32 = mybir.dt.float32
F32R = mybir.dt.float32r
AF = mybir.ActivationFunctionType

N, D, E, H = 16384, 256, 8, 128
NCORES = 8
TPC = N // NCORES          # tokens per core
T = 512                    # token tile (max fp32 moving free dim)
NT = TPC // T              # token tiles per core
KC = D // 128              # 128-row chunks of the model dim

# fused gate/const tensor column layout ([128, GC_W] fp32)
GC_WG1 = 0                 # 256 cols: Wg1 as [p, kc*H + h]
GC_WG2 = 256               # 128 cols: Wg2 replicated into strips 32s+(0..7)
GC_BG1 = 384               # 1 col: bg1
GC_BG2 = 385               # 1 col: bg2 replicated into strips
GC_OH1 = 386               # 3x128 cols: one-hot rows; pass p strip j ->
                           # expert 3p+j (pass 2 only strips 0/1)
GC_OND = 770               # 128 cols: sum-selector; strip s rows have ones
                           # in column 32s only, so the exp-sum matmul's
                           # output lands at partition 32s with col group 0
GC_ON1 = 898               # 128 cols: ones rows at partitions {0,32,64,96}
GC_B2 = 1026               # 256 cols: b2 128-col blocks at strips 0 (mc=0), 1 (mc=1)
GC_B1 = 1282               # 16 cols: b1 as [p, e*KC + kc]
GC_W = 1298

# broadcast pass structure: expert e -> (pass e//3, strip e%3) except
# experts 6,7 -> (pass 2, strips 0,1)
def _pass_strip(e):
    return (e // 3, e % 3) if e < 6 else (2, e - 6)

_CTR = [0]


def _split_multi_waits(nc, max_waits=1):
    """This container's walrus rejects >1 sync-wait per instruction; hoist
    extras onto fresh same-engine NoOps placed just before the waiter."""
    for fn in nc.m.functions:
        for bb in fn.blocks:
            out = []
            for inst in bb.instructions:
                si = inst.sync_info
                waits = list(si.on_wait) if si is not None and si.on_wait else []
                if len(waits) > max_waits:
                    for w in waits[:-max_waits]:
                        _CTR[0] += 1
                        nop = bass_rust.InstNoOp(
                            name=f"I-waitfix-{_CTR[0]}", ins=[], outs=[])
                        nop.engine = inst.engine
                        nop.sync_info = mybir.SyncInfo(on_wait=[w], on_update=[])
                        nc.register_instruction(nop)
                        out.append(nop)
                    si.on_wait = waits[-max_waits:]
                out.append(inst)
            bb.instructions = out


def build_nc(repeat: int = 1):
    nc = bass.Bass("TRN2", target_bir_lowering=False, debug=False,
                   num_devices=NCORES)

    xT_d = nc.dram_tensor("xT", [D, TPC], F32, kind="ExternalInput")
    gc_d = nc.dram_tensor("gc", [128, GC_W], F32, kind="ExternalInput")
    W1_d = nc.dram_tensor("W1", [E, D, D], F32, kind="ExternalInput")
    W2_d = nc.dram_tensor("W2", [E, D, D], F32, kind="ExternalInput")
    yT_d = nc.dram_tensor("yT", [D, TPC], F32, kind="ExternalOutput")

    with tile.TileContext(nc) as tc:
        with (
            nc.allow_low_precision(reason="float32r matmul operands"),
            tc.tile_pool(name="wpool", bufs=1) as wp,
            tc.tile_pool(name="work", bufs=3) as sb,
            tc.tile_pool(name="gbuf", bufs=NT + 1) as gb,
            tc.tile_pool(name="hbuf", bufs=4) as hb,
            tc.tile_pool(name="obuf", bufs=4) as ob,
            tc.tile_pool(name="xpool", bufs=2) as xp,
            tc.tile_pool(name="quad", bufs=3, space="PSUM") as quad,
            tc.tile_pool(name="phid", bufs=3, space="PSUM") as phid,
            tc.tile_pool(name="pout", bufs=2, space="PSUM") as pout,
        ):
            gcx = wp.tile([128, GC_W], F32R, tag="gc")
            nc.sync.dma_start(gcx[:, :], gc_d[:, :].bitcast(F32R))

            def wg1_ap(kc):
                return gcx[:, GC_WG1 + kc * H:GC_WG1 + (kc + 1) * H]
            wg2r = gcx[:, GC_WG2:GC_WG2 + 128]
            bg1 = gcx[:, GC_BG1:GC_BG1 + 1].bitcast(F32)
            bg2r = gcx[:, GC_BG2:GC_BG2 + 1].bitcast(F32)

            def oh_ap(e):
                p, j = _pass_strip(e)
                base = GC_OH1 + 128 * p
                return gcx[32 * j:32 * j + 8, base:base + 128]

            def ond_ap(ti):
                return gcx[32 * ti:32 * ti + 8, GC_OND:GC_OND + 32 * ti + 1]

            def on1_ap(ti):
                return gcx[32 * ti:32 * ti + 1, GC_ON1:GC_ON1 + 128]

            def b2blk(mc):
                return gcx[32 * mc:32 * mc + 8,
                           GC_B2 + 128 * mc:GC_B2 + 128 * (mc + 1)]

            def b1bias(e, mc):
                c = GC_B1 + e * KC + mc
                return gcx[:, c:c + 1].bitcast(F32)

            w1 = wp.tile([128, E, KC, D], F32R, tag="w1")
            w2 = wp.tile([128, E, KC, D], F32R, tag="w2")

            def load_expert_weights(first_only=False, skip_first=False):
                w1src = W1_d.ap().rearrange(
                    "e (kc p) d -> p e kc d", p=128).bitcast(F32R)
                w2src = W2_d.ap().rearrange(
                    "e (kc p) d -> p e kc d", p=128).bitcast(F32R)
                for e in range(1 if skip_first else 0,
                               1 if first_only else E):
                    nc.sync.dma_start(w1[:, e, :, :], w1src[:, e, :, :])
                    nc.sync.dma_start(w2[:, e, :, :], w2src[:, e, :, :])

            def gate(xt, ti, rep, invs):
                """Gate logits/exp for tile ti, gate rows replicated x4
                across strips; exp-sum matmul on strip ti + reciprocal."""
                tok = slice(ti * T, (ti + 1) * T)
                pg1 = quad.tile([128, T], F32, tag="q", name=f"pg1_{rep}_{ti}")
                for kc in range(KC):
                    nc.tensor.matmul(pg1[:, :], wg1_ap(kc), xt[:, kc, tok],
                                     start=(kc == 0), stop=(kc == KC - 1))
                rh = sb.tile([H, T], F32R, tag="rh", name=f"rh_{rep}_{ti}")
                nc.scalar.activation(rh[:, :], pg1[:, :], AF.Relu, bias=bg1)
                pg2 = quad.tile([128, T], F32, tag="q", name=f"pg2_{rep}_{ti}")
                nc.tensor.matmul(pg2[:, :], wg2r, rh[:, :],
                                 start=True, stop=True)
                expl = gb.tile([128, T], F32R, tag="expl",
                               name=f"expl_{rep}_{ti}")
                nc.scalar.activation(expl[:, :], pg2[:, :], AF.Exp, bias=bg2r)
                qs = quad.tile([128, T], F32, tag="q", name=f"qs_{rep}_{ti}")
                nc.tensor.matmul(qs[0:32 * ti + 1, :], ond_ap(ti),
                                 expl[32 * ti:32 * ti + 8, :],
                                 start=True, stop=True,
                                 tile_position=(32 * ti, 0))
                nc.vector.reciprocal(invs[32 * ti:32 * ti + 1, :],
                                     qs[32 * ti:32 * ti + 1, :])
                return expl

            def normalize(ti, rep, invs, expl):
                """Broadcast 1/sum to 128 partitions (K=1 matmul on strip ti)
                and scale the replicated exp rows in place: expl becomes the
                normalized gate, so downstream accumulation needs no final
                renormalization."""
                pv = quad.tile([128, T], F32, tag="q", name=f"pv_{rep}_{ti}")
                nc.tensor.matmul(pv[:, :], on1_ap(ti),
                                 invs[32 * ti:32 * ti + 1, :],
                                 start=True, stop=True,
                                 tile_position=(32 * ti, 0))
                nc.vector.tensor_mul(expl[:, :], expl[:, :], pv[:, :])

            def experts_compute(xt, ti, rep, expl, post_e1=None):
                tok = slice(ti * T, (ti + 1) * T)

                def bcast_pass(base):
                    out = []
                    for e in range(base, min(base + 3, E)):
                        _, j = _pass_strip(e)
                        pt = quad.tile([128, T], F32, tag="q",
                                       name=f"pgb_{rep}_{ti}_{e}")
                        nc.tensor.matmul(pt[:, :], oh_ap(e),
                                         expl[32 * j:32 * j + 8, :],
                                         start=True, stop=True,
                                         tile_position=(32 * j, 0))
                        out.append(pt)
                    return out

                pgb = bcast_pass(0)
                py = None
                for e in range(E):
                    if e in (3, 6):
                        pgb = bcast_pass(e)
                    if e == 2 and post_e1 is not None:
                        post_e1()
                    pt = pgb[e % 3 if e < 6 else e - 6]
                    hs = hb.tile([128, KC, T], F32R, tag="hs",
                                 name=f"hs_{rep}_{ti}_{e}")
                    for mc in range(KC):
                        ph = phid.tile([128, T], F32, tag="ph",
                                       name=f"ph_{rep}_{ti}_{e}_{mc}")
                        for kc in range(KC):
                            nc.tensor.matmul(
                                ph[:, :], w1[:, e, kc, mc * 128:(mc + 1) * 128],
                                xt[:, kc, tok],
                                start=(kc == 0), stop=(kc == KC - 1))
                        nc.scalar.activation(hs[:, mc, :], ph[:, :], AF.Relu,
                                             bias=b1bias(e, mc))
                        nc.vector.tensor_mul(hs[:, mc, :], hs[:, mc, :],
                                             pt[:, :])
                    if e == 0:
                        # b2 init after e0's first-layer matmuls: gives the
                        # previous tile's output copies time to free the
                        # banks before the accumulation group opens.
                        py = [pout.tile([128, T], F32, tag="py",
                                        name=f"py{mc}_{rep}_{ti}")
                              for mc in range(KC)]
                        for mc in range(KC):
                            nc.tensor.matmul(py[mc][:, :], b2blk(mc),
                                             expl[32 * mc:32 * mc + 8, :],
                                             start=True, stop=False,
                                             tile_position=(32 * mc, 0))
                    for mc in range(KC):
                        for kc in range(KC):
                            nc.tensor.matmul(
                                py[mc][:, :],
                                w2[:, e, kc, mc * 128:(mc + 1) * 128],
                                hs[:, kc, :],
                                start=False,
                                stop=(e == E - 1 and kc == KC - 1))
                return py

            def finalize(ti, rep, py):
                tok = slice(ti * T, (ti + 1) * T)
                for mc in range(KC):
                    ot = ob.tile([128, T], F32, tag="ot",
                                 name=f"ot_{rep}_{ti}_{mc}")
                    nc.scalar.activation(ot[:, :], py[mc][:, :], AF.Copy)
                    nc.gpsimd.dma_start(yT_d[mc * 128:(mc + 1) * 128, tok],
                                        ot[:, :])

            for rep in range(repeat):
                xt = xp.tile([128, KC, TPC], F32R, tag="xt", name=f"xt{rep}")
                xsrc = xT_d.ap().rearrange(
                    "(kc p) t -> p kc t", p=128).bitcast(F32R)
                nc.sync.dma_start(xt[:, :, 0:T], xsrc[:, :, 0:T])
                if rep == 0:
                    load_expert_weights(first_only=True)
                for ti in range(1, NT):
                    tok = slice(ti * T, (ti + 1) * T)
                    nc.sync.dma_start(xt[:, :, tok], xsrc[:, :, tok])
                invs = sb.tile([128, T], F32R, tag="invs", name=f"invs_{rep}")
                expls = []
                for ti in range(NT):
                    expls.append(gate(xt, ti, rep, invs))
                    # normalize lags one tile behind the gate so the
                    # reciprocal is ready and the PE never waits on it
                    if ti >= 1:
                        normalize(ti - 1, rep, invs, expls[ti - 1])
                if rep == 0:
                    load_expert_weights(skip_first=True)
                for ti in range(NT):
                    # the last tile's normalize is deferred into tile 0's
                    # expert phase so its reciprocal has time to finish and
                    # its PSUM bank never gates the broadcast-pass ring
                    post = ((lambda: normalize(NT - 1, rep, invs,
                                               expls[NT - 1]))
                            if ti == 0 else None)
                    py = experts_compute(xt, ti, rep, expls[ti],
                                         post_e1=post)
                    finalize(ti, rep, py)

    _split_multi_waits(nc)
    return nc


_NC_CACHE = None


def _get_nc():
    global _NC_CACHE
    if _NC_CACHE is None:
        _NC_CACHE = build_nc()
    return _NC_CACHE


def make_in_maps(x, Wg1, bg1, Wg2, bg2, W1, b1, W2, b2):
    x = np.ascontiguousarray(np.asarray(x, dtype=np.float32))
    xT = np.ascontiguousarray(x.T)           # [D, N]
    Wg1 = np.asarray(Wg1, np.float32)
    bg1 = np.asarray(bg1, np.float32)
    Wg2 = np.asarray(Wg2, np.float32)
    bg2 = np.asarray(bg2, np.float32)
    b1 = np.asarray(b1, np.float32)
    b2 = np.asarray(b2, np.float32)

    gc = np.zeros((128, GC_W), np.float32)
    # Wg1 [D, H] -> [p, kc*H + h]
    gc[:, GC_WG1:GC_WG1 + KC * H] = (
        Wg1.reshape(KC, 128, H).transpose(1, 0, 2).reshape(128, KC * H))
    # Wg2 replicated: wg2r[h, 32s+k] = Wg2[h, k]; bg2 likewise per strip
    for s in range(4):
        gc[:, GC_WG2 + 32 * s:GC_WG2 + 32 * s + 8] = Wg2
        gc[32 * s:32 * s + 8, GC_BG2] = bg2
    gc[:, GC_BG1] = bg1
    # one-hot strips: expert e lives in pass e//3 at strip e%3 (6,7: pass 2
    # strips 0/1); within its [8,128] strip block, row e is ones
    for e in range(E):
        p, j = _pass_strip(e)
        gc[32 * j + e, GC_OH1 + 128 * p:GC_OH1 + 128 * (p + 1)] = 1.0
    for j in range(4):
        gc[32 * j:32 * j + 8, GC_OND + 32 * j] = 1.0
        gc[32 * j, GC_ON1:GC_ON1 + 128] = 1.0
    # b2 blocks: strip mc holds b2[:, mc*128:(mc+1)*128]
    for mc in range(KC):
        gc[32 * mc:32 * mc + 8,
           GC_B2 + 128 * mc:GC_B2 + 128 * (mc + 1)] = b2[:, mc * 128:(mc + 1) * 128]
    # b1 as [p, e*KC + kc]
    gc[:, GC_B1:GC_B1 + E * KC] = (
        b1.reshape(E, KC, 128).transpose(2, 0, 1).reshape(128, E * KC))

    shared = {
        "gc": np.ascontiguousarray(gc),
        "W1": np.ascontiguousarray(np.asarray(W1, np.float32)),
        "W2": np.ascontiguousarray(np.asarray(W2, np.float32)),
    }
    return [
        {"xT": np.ascontiguousarray(xT[:, c * TPC:(c + 1) * TPC]), **shared}
        for c in range(NCORES)
    ]


def gather_output(results):
    out = np.empty((N, D), np.float32)
    for c in range(NCORES):
        out[c * TPC:(c + 1) * TPC, :] = results[c]["yT"].T
    return out


def kernel(x, Wg1, bg1, Wg2, bg2, W1, b1, W2, b2):
    nc = _get_nc()
    in_maps = make_in_maps(x, Wg1, bg1, Wg2, bg2, W1, b1, W2, b2)
    r = run_bass_kernel_spmd(nc, in_maps, list(range(NCORES)))
    return gather_output(r.results)



# revision 35
# speedup vs baseline: 1.3961x; 1.3961x over previous
"""MoE (dense routing) Trainium2 kernel: 8-core data-parallel over tokens.

Problem: nn_MixtureOfExperts_33011118637071
  N=16384 tokens, D=256 model dim, E=8 experts, H=128 gate hidden.
  gate   = softmax(relu(x @ Wg1 + bg1) @ Wg2 + bg2)          [N, E]
  h_e    = relu(x @ W1[e] + b1[e])                           [N, D]
  y      = sum_e gate[:, e] * (h_e @ W2[e] + b2[e])          [N, D]

Strategy (per core, 2048 tokens):
  Feature-major layout (features on partitions, tokens on the free dim) so
  the two expert GEMMs chain without transposes; x is transposed on the
  host as part of sharding and the output transposed back on gather.

  v6 over v2 (92.5us -> ~66us under the TRN2 cost model):
  - All matmul operands are bf16 (same PE rate as float32r at free>=256,
    half the HBM traffic; rel err ~4e-3, well under the 2e-2 gate).
  - The gate-row broadcasts, which v2 did as K<=8 PE matmuls packed with
    tile_position, move off the PE entirely onto the DMA engines via a
    DRAM bounce: per tile, the 8 raw exp rows + the 1/sum row are written
    to a scratch DRAM buffer (tiny), then one DMA with a zero-stride
    source reads them back replicated to all 128 partitions as
    gall[128, 9, T] (~3.3us of DMA, fully off the PE). GPSIMD
    partition_broadcast would be cheaper but does not survive walrus
    codegen ("ISA wrong length"), and SBUF-source DMAs reject zero
    partition stride — DRAM-source broadcast is the one path that
    compiles. The PE then only runs real GEMM passes: per tile 64 expert
    matmuls + 2 pg1 + pg2 + exp-sum + 2 b2-init = 70 passes x 512 rows
    ~ 14.9us/tile.
  - Softmax normalization is deferred to the very end: experts accumulate
    exp-weighted (unnormalized) outputs in PSUM — the b2 term rides along
    as sum_e exp_e*b2_e — and the output evacuation multiplies by the
    replicated 1/sum row on DVE. An earlier variant that normalized the
    gate rows in SBUF head-of-line blocked the in-order DVE queue behind
    the broadcast backlog.
  - Gate-multiply on DVE reads bf16 SBUF operands only -> 2x DVE mode
    (327ns vs 658ns for the v2 PSUM-operand form).
  - The gate stages are software-pipelined across token tiles
    (pg1/relu | pg2/exp | sum/recip/bounce) so the PE streams gate
    matmuls back-to-back instead of waiting on each tile's serial
    ScalarE->PE->ScalarE chain.
  - Expert layer-2 emission lags layer-1 by one expert (l1(e+1) issues
    before l2(e)) so the relu->gate-multiply chain of expert e hides
    under expert e+1's layer-1 passes; within l2 the kc=1 operands (whose
    relu finishes last) are consumed after both kc=0 passes; the b2 PSUM
    init sits at e==1, giving the previous tile's output evacuations
    ~1.7us to free the banks.
  - W1/W2/x are host-packed into their exact SBUF layouts (trivial DMA
    descriptors). All x/weight/broadcast traffic rides the SP queue,
    interleaved in compute-need order (the DMA engines are modeled as a
    serial resource, so issue order is arrival order); output stores ride
    the Pool queue (SWDGE) to keep the Activation SEQ free of its 667ns
    per-DMA issue cost.
  - PSUM: 3 gate banks + 3 hidden banks + 2 output-accum banks = 8.
"""
import numpy as np
import ml_dtypes

import bass_rust
import concourse.bass as bass
import concourse.mybir as mybir
import concourse.tile as tile
from concourse.bass_utils import run_bass_kernel_spmd

F32 = mybir.dt.float32
BF16 = mybir.dt.bfloat16
AF = mybir.ActivationFunctionType

N, D, E, H = 16384, 256, 8, 128
NCORES = 8
TPC = N // NCORES          # tokens per core
T = 512                    # token tile (max fp32 PSUM moving free dim)
NT = TPC // T              # token tiles per core
KC = D // 128              # 128-row chunks of the model dim

# bf16 matmul-const tensor column layout ([128, GB_W])
GB_WG1 = 0                 # 256 cols: Wg1 as [p, kc*H + h]
GB_WG2 = 256               # 128 cols: Wg2 replicated into strips 32s+(0..7)
GB_OND = 384               # 1 col: exp-sum selector; rows 32s+(0..7) are 1,
                           # so each strip's matmul sums its 8 exp rows into
                           # output partition 0
GB_B2 = 385                # 256 cols: b2 128-col blocks at strips 0/1
GB_W = 641

# f32 bias tensor column layout ([128, GF_W])
GF_BG1 = 0                 # bg1
GF_BG2 = 1                 # bg2 replicated into strips
GF_B1 = 2                  # 16 cols: b1 as [p, e*KC + kc]
GF_W = 18

_CTR = [0]


def _split_multi_waits(nc, max_waits=1):
    """This container's walrus rejects >1 sync-wait per instruction; hoist
    extras onto fresh same-engine NoOps placed just before the waiter."""
    for fn in nc.m.functions:
        for bb in fn.blocks:
            out = []
            for inst in bb.instructions:
                si = inst.sync_info
                waits = list(si.on_wait) if si is not None and si.on_wait else []
                if len(waits) > max_waits:
                    for w in waits[:-max_waits]:
                        _CTR[0] += 1
                        nop = bass_rust.InstNoOp(
                            name=f"I-waitfix-{_CTR[0]}", ins=[], outs=[])
                        nop.engine = inst.engine
                        nop.sync_info = mybir.SyncInfo(on_wait=[w], on_update=[])
                        nc.register_instruction(nop)
                        out.append(nop)
                    si.on_wait = waits[-max_waits:]
                out.append(inst)
            bb.instructions = out


def build_nc(repeat: int = 1):
    nc = bass.Bass("TRN2", target_bir_lowering=False, debug=False,
                   num_devices=NCORES)

    # all big tensors host-packed into their SBUF layouts (trivial DMAs)
    x_d = nc.dram_tensor("xs", [128, KC, TPC], BF16, kind="ExternalInput")
    gb_d = nc.dram_tensor("gb", [128, GB_W], BF16, kind="ExternalInput")
    gf_d = nc.dram_tensor("gf", [128, GF_W], F32, kind="ExternalInput")
    W1_d = nc.dram_tensor("W1", [128, E, KC, D], BF16, kind="ExternalInput")
    W2_d = nc.dram_tensor("W2", [128, E, KC, D], BF16, kind="ExternalInput")
    yT_d = nc.dram_tensor("yT", [D, TPC], BF16, kind="ExternalOutput")
    # per-tile DRAM scratch for the gate-row broadcast bounce; all access
    # rides the in-order SP queue, so cross-rep reuse is write-after-read
    # safe by queue order
    gd_d = [nc.dram_tensor(f"gd{ti}", [E + 1, T], BF16, kind="Internal")
            for ti in range(NT)]

    with tile.TileContext(nc) as tc:
        with (
            nc.allow_low_precision(reason="bf16 matmul operands"),
            tc.tile_pool(name="wpool", bufs=1) as wp,
            tc.tile_pool(name="work", bufs=3) as sb,
            tc.tile_pool(name="gbuf", bufs=NT + 1) as gb,
            tc.tile_pool(name="hbuf", bufs=4) as hb,
            tc.tile_pool(name="obuf", bufs=4) as ob,
            tc.tile_pool(name="xpool", bufs=2) as xp,
            tc.tile_pool(name="gall", bufs=NT + 1) as ga,
            tc.tile_pool(name="quad", bufs=3, space="PSUM") as quad,
            tc.tile_pool(name="phid", bufs=3, space="PSUM") as phid,
            tc.tile_pool(name="pout", bufs=2, space="PSUM") as pout,
        ):
            w1 = wp.tile([128, E, KC, D], BF16, tag="w1")
            w2 = wp.tile([128, E, KC, D], BF16, tag="w2")

            gbx = wp.tile([128, GB_W], BF16, tag="gb")
            gfx = wp.tile([128, GF_W], F32, tag="gf")
            nc.scalar.dma_start(gbx[:, :], gb_d[:, :])
            nc.scalar.dma_start(gfx[:, :], gf_d[:, :])

            def load_weights(es):
                # per-expert transfers on the Pool/SWDGE queue: descriptor
                # generation paces them ~1us apart from t~0, so they drip
                # into the serial DMA resource in need order without
                # front-running the gate-phase x/bounce traffic on SP
                for e in es:
                    nc.gpsimd.dma_start(w1[:, e, :, :], W1_d[:, e, :, :])
                    nc.gpsimd.dma_start(w2[:, e, :, :], W2_d[:, e, :, :])

            def wg1_ap(kc):
                return gbx[:, GB_WG1 + kc * H:GB_WG1 + (kc + 1) * H]
            wg2r = gbx[:, GB_WG2:GB_WG2 + 128]
            bg1 = gfx[:, GF_BG1:GF_BG1 + 1]
            bg2r = gfx[:, GF_BG2:GF_BG2 + 1]

            def ond_ap(ti):
                return gbx[32 * ti:32 * ti + 8, GB_OND:GB_OND + 1]

            def b2blk(mc):
                return gbx[32 * mc:32 * mc + 8,
                           GB_B2 + 128 * mc:GB_B2 + 128 * (mc + 1)]

            def b1bias(e, mc):
                c = GF_B1 + e * KC + mc
                return gfx[:, c:c + 1]

            def gate_a(xt, ti, rep):
                """pg1 matmuls + relu -> gate hidden rh."""
                tok = slice(ti * T, (ti + 1) * T)
                pg1 = quad.tile([128, T], F32, tag="q", name=f"pg1_{rep}_{ti}")
                for kc in range(KC):
                    nc.tensor.matmul(pg1[:, :], wg1_ap(kc), xt[:, kc, tok],
                                     start=(kc == 0), stop=(kc == KC - 1))
                rh = sb.tile([H, T], BF16, tag="rh", name=f"rh_{rep}_{ti}")
                nc.scalar.activation(rh[:, :], pg1[:, :], AF.Relu, bias=bg1)
                return rh

            def gate_b(rh, ti, rep):
                """pg2 matmul + exp -> replicated raw-exp rows expl."""
                pg2 = quad.tile([128, T], F32, tag="q", name=f"pg2_{rep}_{ti}")
                nc.tensor.matmul(pg2[:, :], wg2r, rh[:, :],
                                 start=True, stop=True)
                expl = gb.tile([128, T], BF16, tag="expl",
                               name=f"expl_{rep}_{ti}")
                nc.scalar.activation(expl[:, :], pg2[:, :], AF.Exp, bias=bg2r)
                return expl

            def gate_c(expl, ti, rep):
                """exp-sum matmul (strip ti -> output partition 0),
                reciprocal, then the broadcast bounce: raw exp rows + the
                1/sum row go to DRAM and come back replicated to all 128
                partitions as gall[128, E+1, T]."""
                qs = quad.tile([128, T], F32, tag="q", name=f"qs_{rep}_{ti}")
                nc.tensor.matmul(qs[0:1, :], ond_ap(ti),
                                 expl[32 * ti:32 * ti + 8, :],
                                 start=True, stop=True,
                                 tile_position=(32 * ti, 0))
                invr = sb.tile([1, T], BF16, tag="invr",
                               name=f"invr_{rep}_{ti}")
                nc.vector.reciprocal(invr[0:1, :], qs[0:1, :])
                # DRAM tensors are not dependency-tracked by the tile
                # framework: chain every gd access (sync=true) so the
                # replicated reads follow the row writes (RAW) and the next
                # rep's row writes follow this rep's reads (WAR)
                key = f"gd{ti}"

                def chain(inst):
                    tc.chain_iter_dep(key, inst.ins)

                chain(nc.sync.dma_start(gd_d[ti][0:E, :], expl[0:E, :]))
                chain(nc.sync.dma_start(gd_d[ti][E:E + 1, :], invr[0:1, :]))
                gall = ga.tile([128, E + 1, T], BF16, tag="gall",
                               name=f"gall_{rep}_{ti}")
                src = gd_d[ti].ap().unsqueeze(0)
                if rep == 0:
                    # rep 0 contends with the weight stream on the serial
                    # DMA resource: two halves let experts 0..3 unblock
                    # ~1.6us earlier than a monolithic replicated read
                    chain(nc.sync.dma_start(
                        gall[:, 0:4, :],
                        src[:, 0:4, :].broadcast_to([128, 4, T])))
                    chain(nc.sync.dma_start(
                        gall[:, 4:E + 1, :],
                        src[:, 4:E + 1, :].broadcast_to([128, E + 1 - 4, T])))
                else:
                    chain(nc.sync.dma_start(
                        gall[:, :, :], src.broadcast_to([128, E + 1, T])))
                return gall

            def experts_compute(xt, ti, rep, expl, gall):
                """Layer-2 emission lags layer-1 by 1.5 experts (l2(e)
                issues between l1(e+2,mc0) and l1(e+2,mc1)) so each
                expert's relu->gate-multiply chain (~1.4us) hides under
                ~1.7us of later layer-1 passes; b2 init at e==1 gives the
                previous tile's output evacuations time to free the pout
                banks."""
                tok = slice(ti * T, (ti + 1) * T)
                py = None
                hss = [None] * E

                def emit_l2(e):
                    # kc outer: the kc=1 operand's relu finishes last, so
                    # both kc=0 passes run first and buy it ~426ns
                    for kc in range(KC):
                        for mc in range(KC):
                            nc.tensor.matmul(
                                py[mc][:, :],
                                w2[:, e, kc, mc * 128:(mc + 1) * 128],
                                hss[e][:, kc, :],
                                start=False,
                                stop=(e == E - 1 and kc == KC - 1))

                for e in range(E):
                    pt = gall[:, e, :]
                    hs = hb.tile([128, KC, T], BF16, tag="hs",
                                 name=f"hs_{rep}_{ti}_{e}")
                    hss[e] = hs
                    for mc in range(KC):
                        ph = phid.tile([128, T], F32, tag="ph",
                                       name=f"ph_{rep}_{ti}_{e}_{mc}")
                        for kc in range(KC):
                            nc.tensor.matmul(
                                ph[:, :], w1[:, e, kc, mc * 128:(mc + 1) * 128],
                                xt[:, kc, tok],
                                start=(kc == 0), stop=(kc == KC - 1))
                        nc.scalar.activation(hs[:, mc, :], ph[:, :], AF.Relu,
                                             bias=b1bias(e, mc))
                        nc.vector.tensor_mul(hs[:, mc, :], hs[:, mc, :],
                                             pt[:, :])
                        if mc == 0 and e >= 2:
                            emit_l2(e - 2)
                    if e == 1:
                        py = [pout.tile([128, T], F32, tag="py",
                                        name=f"py{mc}_{rep}_{ti}")
                              for mc in range(KC)]
                        for mc in range(KC):
                            nc.tensor.matmul(py[mc][:, :], b2blk(mc),
                                             expl[32 * mc:32 * mc + 8, :],
                                             start=True, stop=False,
                                             tile_position=(32 * mc, 0))
                emit_l2(E - 2)
                emit_l2(E - 1)
                return py

            def finalize(ti, rep, py, gall):
                tok = slice(ti * T, (ti + 1) * T)
                for mc in range(KC):
                    ot = ob.tile([128, T], BF16, tag="ot",
                                 name=f"ot_{rep}_{ti}_{mc}")
                    nc.vector.tensor_mul(ot[:, :], py[mc][:, :],
                                         gall[:, E, :])
                    nc.gpsimd.dma_start(yT_d[mc * 128:(mc + 1) * 128, tok],
                                        ot[:, :])

            load_weights(range(E))

            for rep in range(repeat):
                xt = xp.tile([128, KC, TPC], BF16, tag="xt", name=f"xt{rep}")
                for ti in range(NT):
                    tok = slice(ti * T, (ti + 1) * T)
                    nc.sync.dma_start(xt[:, :, tok], x_d[:, :, tok])

                # software-pipelined gate: A=pg1/relu, B=pg2/exp,
                # C=sum/recip/bounce; stage k of tile ti issues while
                # stage k+1 of tile ti-1 is still in flight. Weight loads
                # weave between the per-tile bounce DMAs so the serial DMA
                # resource serves everything in compute-need order.
                rhs = [None] * NT
                expls = [None] * NT
                galls = [None] * NT
                rhs[0] = gate_a(xt, 0, rep)
                rhs[1] = gate_a(xt, 1, rep)
                expls[0] = gate_b(rhs[0], 0, rep)
                rhs[2] = gate_a(xt, 2, rep)
                expls[1] = gate_b(rhs[1], 1, rep)
                galls[0] = gate_c(expls[0], 0, rep)
                rhs[3] = gate_a(xt, 3, rep)
                expls[2] = gate_b(rhs[2], 2, rep)
                galls[1] = gate_c(expls[1], 1, rep)
                expls[3] = gate_b(rhs[3], 3, rep)
                galls[2] = gate_c(expls[2], 2, rep)
                galls[3] = gate_c(expls[3], 3, rep)

                for ti in range(NT):
                    py = experts_compute(xt, ti, rep, expls[ti], galls[ti])
                    finalize(ti, rep, py, galls[ti])

    _split_multi_waits(nc)
    return nc


_NC_CACHE = None


def _get_nc():
    global _NC_CACHE
    if _NC_CACHE is None:
        _NC_CACHE = build_nc()
    return _NC_CACHE


def make_in_maps(x, Wg1, bg1, Wg2, bg2, W1, b1, W2, b2):
    bf = ml_dtypes.bfloat16
    x = np.ascontiguousarray(np.asarray(x, dtype=np.float32))
    Wg1 = np.asarray(Wg1, np.float32)
    bg1 = np.asarray(bg1, np.float32)
    Wg2 = np.asarray(Wg2, np.float32)
    bg2 = np.asarray(bg2, np.float32)
    W1 = np.asarray(W1, np.float32)
    b1 = np.asarray(b1, np.float32)
    W2 = np.asarray(W2, np.float32)
    b2 = np.asarray(b2, np.float32)

    gcb = np.zeros((128, GB_W), np.float32)
    gcf = np.zeros((128, GF_W), np.float32)
    # Wg1 [D, H] -> [p, kc*H + h]
    gcb[:, GB_WG1:GB_WG1 + KC * H] = (
        Wg1.reshape(KC, 128, H).transpose(1, 0, 2).reshape(128, KC * H))
    # Wg2 replicated: wg2r[h, 32s+k] = Wg2[h, k]; bg2 likewise per strip
    for s in range(4):
        gcb[:, GB_WG2 + 32 * s:GB_WG2 + 32 * s + 8] = Wg2
        gcf[32 * s:32 * s + 8, GF_BG2] = bg2
    gcf[:, GF_BG1] = bg1
    for j in range(4):
        gcb[32 * j:32 * j + 8, GB_OND] = 1.0
    # b2 blocks: strip mc holds b2[:, mc*128:(mc+1)*128]
    for mc in range(KC):
        gcb[32 * mc:32 * mc + 8,
            GB_B2 + 128 * mc:GB_B2 + 128 * (mc + 1)] = b2[:, mc * 128:(mc + 1) * 128]
    # b1 as [p, e*KC + kc]
    gcf[:, GF_B1:GF_B1 + E * KC] = (
        b1.reshape(E, KC, 128).transpose(2, 0, 1).reshape(128, E * KC))

    # SBUF layouts, host-packed:
    #   x:  [N, D] -> xT [D=(kc p), N] -> [p, kc, n]
    xs = np.ascontiguousarray(
        x.T.reshape(KC, 128, N).transpose(1, 0, 2).astype(bf))
    #   W:  [E, D=(kc p), D] -> [p, e, kc, d]
    w1s = np.ascontiguousarray(
        W1.reshape(E, KC, 128, D).transpose(2, 0, 1, 3).astype(bf))
    w2s = np.ascontiguousarray(
        W2.reshape(E, KC, 128, D).transpose(2, 0, 1, 3).astype(bf))

    shared = {
        "gb": np.ascontiguousarray(gcb.astype(bf)),
        "gf": np.ascontiguousarray(gcf),
        "W1": w1s,
        "W2": w2s,
    }
    return [
        {"xs": np.ascontiguousarray(xs[:, :, c * TPC:(c + 1) * TPC]), **shared}
        for c in range(NCORES)
    ]


def gather_output(results):
    out = np.empty((N, D), np.float32)
    for c in range(NCORES):
        out[c * TPC:(c + 1) * TPC, :] = (
            np.asarray(results[c]["yT"]).astype(np.float32).T)
    return out


def kernel(x, Wg1, bg1, Wg2, bg2, W1, b1, W2, b2):
    nc = _get_nc()
    in_maps = make_in_maps(x, Wg1, bg1, Wg2, bg2, W1, b1, W2, b2)
    r = run_bass_kernel_spmd(nc, in_maps, list(range(NCORES)))
    return gather_output(r.results)


# revision 48
# speedup vs baseline: 1.4357x; 1.0284x over previous
"""MoE (dense routing) Trainium2 kernel: 8-core data-parallel over tokens.

Problem: nn_MixtureOfExperts_33011118637071
  N=16384 tokens, D=256 model dim, E=8 experts, H=128 gate hidden.
  gate   = softmax(relu(x @ Wg1 + bg1) @ Wg2 + bg2)          [N, E]
  h_e    = relu(x @ W1[e] + b1[e])                           [N, D]
  y      = sum_e gate[:, e] * (h_e @ W2[e] + b2[e])          [N, D]

Strategy (per core, 2048 tokens):
  Feature-major layout (features on partitions, tokens on the free dim) so
  the two expert GEMMs chain without transposes; x is transposed on the
  host as part of sharding and the output transposed back on gather.

  v6 over v2 (92.5us -> ~66us under the TRN2 cost model):
  - All matmul operands are bf16 (same PE rate as float32r at free>=256,
    half the HBM traffic; rel err ~4e-3, well under the 2e-2 gate).
  - The gate-row broadcasts, which v2 did as K<=8 PE matmuls packed with
    tile_position, move off the PE entirely onto the DMA engines via a
    DRAM bounce: per tile, the 8 raw exp rows + the 1/sum row are written
    to a scratch DRAM buffer (tiny), then one DMA with a zero-stride
    source reads them back replicated to all 128 partitions as
    gall[128, 9, T] (~3.3us of DMA, fully off the PE). GPSIMD
    partition_broadcast would be cheaper but does not survive walrus
    codegen ("ISA wrong length"), and SBUF-source DMAs reject zero
    partition stride — DRAM-source broadcast is the one path that
    compiles. The PE then only runs real GEMM passes: per tile 64 expert
    matmuls + 2 pg1 + pg2 + exp-sum + 2 b2-init = 70 passes x 512 rows
    ~ 14.9us/tile.
  - Softmax normalization is deferred to the very end: experts accumulate
    exp-weighted (unnormalized) outputs in PSUM — the b2 term rides along
    as sum_e exp_e*b2_e — and the output evacuation multiplies by the
    replicated 1/sum row on DVE. An earlier variant that normalized the
    gate rows in SBUF head-of-line blocked the in-order DVE queue behind
    the broadcast backlog.
  - Gate-multiply on DVE reads bf16 SBUF operands only -> 2x DVE mode
    (327ns vs 658ns for the v2 PSUM-operand form).
  - The gate stages are software-pipelined across token tiles
    (pg1/relu | pg2/exp | sum/recip/bounce) so the PE streams gate
    matmuls back-to-back instead of waiting on each tile's serial
    ScalarE->PE->ScalarE chain.
  - Expert layer-2 emission lags layer-1 by one expert (l1(e+1) issues
    before l2(e)) so the relu->gate-multiply chain of expert e hides
    under expert e+1's layer-1 passes; within l2 the kc=1 operands (whose
    relu finishes last) are consumed after both kc=0 passes; the b2 PSUM
    init sits at e==1, giving the previous tile's output evacuations
    ~1.7us to free the banks.
  - W1/W2/x are host-packed into their exact SBUF layouts (trivial DMA
    descriptors). x + bounce traffic rides the SP queue in compute-need
    order; the rep-0 weight stream rides the Pool/SWDGE queue whose
    generation naturally paces it; output stores also ride the Pool queue
    to keep the Activation SEQ free of its 667ns per-DMA issue cost.
  - DMA instruction COUNT matters on the measured backend (~0.7-1us of
    unmodeled per-DMA overhead): x loads are 2 DMAs per rep (tile 0 +
    rest), each tile's output is a single [128, KC, T] store (4/rep), and
    the 1/sum row rides inside the same bounce as the gate rows. Moving
    the 1/sum replication to a K=1 PE pass + ScalarE evacuation measured
    WORSE (81.5us vs 67.6us) despite saving a DMA - the extra quad-bank
    pressure and PE/ScalarE serialization cost more than the DMA.
  - PSUM: 3 gate banks + 3 hidden banks + 2 output-accum banks = 8.
"""
import numpy as np
import ml_dtypes

import bass_rust
import concourse.bass as bass
import concourse.mybir as mybir
import concourse.tile as tile
from concourse.bass_utils import run_bass_kernel_spmd

F32 = mybir.dt.float32
BF16 = mybir.dt.bfloat16
AF = mybir.ActivationFunctionType

N, D, E, H = 16384, 256, 8, 128
NCORES = 8
TPC = N // NCORES          # tokens per core
T = 512                    # token tile (max fp32 PSUM moving free dim)
NT = TPC // T              # token tiles per core
KC = D // 128              # 128-row chunks of the model dim

# bf16 matmul-const tensor column layout ([128, GB_W])
GB_WG1 = 0                 # 256 cols: Wg1 as [p, kc*H + h]
GB_WG2 = 256               # 128 cols: Wg2 replicated into strips 32s+(0..7)
GB_OND = 384               # 1 col: exp-sum selector; rows 32s+(0..7) are 1,
                           # so each strip's matmul sums its 8 exp rows into
                           # output partition 0
GB_B2 = 385                # 256 cols: b2 128-col blocks at strips 0/1
GB_ON1 = 641               # 128 cols: ones row at partition 0 (1/sum K=1
                           # broadcast matmul)
GB_W = 769

# f32 bias tensor column layout ([128, GF_W])
GF_BG1 = 0                 # bg1
GF_BG2 = 1                 # bg2 replicated into strips
GF_B1 = 2                  # 16 cols: b1 as [p, e*KC + kc]
GF_W = 18

_CTR = [0]


def _split_multi_waits(nc, max_waits=1):
    """This container's walrus rejects >1 sync-wait per instruction; hoist
    extras onto fresh same-engine NoOps placed just before the waiter."""
    for fn in nc.m.functions:
        for bb in fn.blocks:
            out = []
            for inst in bb.instructions:
                si = inst.sync_info
                waits = list(si.on_wait) if si is not None and si.on_wait else []
                if len(waits) > max_waits:
                    for w in waits[:-max_waits]:
                        _CTR[0] += 1
                        nop = bass_rust.InstNoOp(
                            name=f"I-waitfix-{_CTR[0]}", ins=[], outs=[])
                        nop.engine = inst.engine
                        nop.sync_info = mybir.SyncInfo(on_wait=[w], on_update=[])
                        nc.register_instruction(nop)
                        out.append(nop)
                    si.on_wait = waits[-max_waits:]
                out.append(inst)
            bb.instructions = out


def build_nc(repeat: int = 1):
    nc = bass.Bass("TRN2", target_bir_lowering=False, debug=False,
                   num_devices=NCORES)

    # all big tensors host-packed into their SBUF layouts (trivial DMAs)
    x_d = nc.dram_tensor("xs", [128, KC, TPC], BF16, kind="ExternalInput")
    gb_d = nc.dram_tensor("gb", [128, GB_W], BF16, kind="ExternalInput")
    gf_d = nc.dram_tensor("gf", [128, GF_W], F32, kind="ExternalInput")
    W1_d = nc.dram_tensor("W1", [128, E, KC, D], BF16, kind="ExternalInput")
    W2_d = nc.dram_tensor("W2", [128, E, KC, D], BF16, kind="ExternalInput")
    yT_d = nc.dram_tensor("yT", [D, TPC], BF16, kind="ExternalOutput")
    # per-tile DRAM scratch for the gate-row broadcast bounce
    gd_d = [nc.dram_tensor(f"gd{ti}", [E + 1, T], BF16, kind="Internal")
            for ti in range(NT)]

    with tile.TileContext(nc) as tc:
        with (
            nc.allow_low_precision(reason="bf16 matmul operands"),
            tc.tile_pool(name="wpool", bufs=1) as wp,
            tc.tile_pool(name="work", bufs=3) as sb,
            tc.tile_pool(name="gbuf", bufs=NT + 1) as gb,
            tc.tile_pool(name="hbuf", bufs=4) as hb,
            tc.tile_pool(name="obuf", bufs=4) as ob,
            tc.tile_pool(name="xpool", bufs=2) as xp,
            tc.tile_pool(name="gall", bufs=NT + 1) as ga,
            tc.tile_pool(name="quad", bufs=3, space="PSUM") as quad,
            tc.tile_pool(name="phid", bufs=3, space="PSUM") as phid,
            tc.tile_pool(name="pout", bufs=2, space="PSUM") as pout,
        ):
            w1 = wp.tile([128, E, KC, D], BF16, tag="w1")
            w2 = wp.tile([128, E, KC, D], BF16, tag="w2")

            gbx = wp.tile([128, GB_W], BF16, tag="gb")
            gfx = wp.tile([128, GF_W], F32, tag="gf")
            nc.scalar.dma_start(gbx[:, :], gb_d[:, :])
            nc.scalar.dma_start(gfx[:, :], gf_d[:, :])

            def load_weights(es):
                # per-expert transfers on the Pool/SWDGE queue: descriptor
                # generation paces them ~1us apart from t~0, so they drip
                # into the serial DMA resource in need order without
                # front-running the gate-phase x/bounce traffic on SP
                for e in es:
                    nc.gpsimd.dma_start(w1[:, e, :, :], W1_d[:, e, :, :])
                    nc.gpsimd.dma_start(w2[:, e, :, :], W2_d[:, e, :, :])

            def wg1_ap(kc):
                return gbx[:, GB_WG1 + kc * H:GB_WG1 + (kc + 1) * H]
            wg2r = gbx[:, GB_WG2:GB_WG2 + 128]
            bg1 = gfx[:, GF_BG1:GF_BG1 + 1]
            bg2r = gfx[:, GF_BG2:GF_BG2 + 1]

            def ond_ap(ti):
                return gbx[32 * ti:32 * ti + 8, GB_OND:GB_OND + 1]

            def b2blk(mc):
                return gbx[32 * mc:32 * mc + 8,
                           GB_B2 + 128 * mc:GB_B2 + 128 * (mc + 1)]

            def b1bias(e, mc):
                c = GF_B1 + e * KC + mc
                return gfx[:, c:c + 1]

            def gate_a(xt, ti, rep):
                """pg1 matmuls + relu -> gate hidden rh."""
                tok = slice(ti * T, (ti + 1) * T)
                pg1 = quad.tile([128, T], F32, tag="q", name=f"pg1_{rep}_{ti}")
                for kc in range(KC):
                    nc.tensor.matmul(pg1[:, :], wg1_ap(kc), xt[:, kc, tok],
                                     start=(kc == 0), stop=(kc == KC - 1))
                rh = sb.tile([H, T], BF16, tag="rh", name=f"rh_{rep}_{ti}")
                nc.scalar.activation(rh[:, :], pg1[:, :], AF.Relu, bias=bg1)
                return rh

            def gate_b(rh, ti, rep):
                """pg2 matmul + exp -> replicated raw-exp rows expl."""
                pg2 = quad.tile([128, T], F32, tag="q", name=f"pg2_{rep}_{ti}")
                nc.tensor.matmul(pg2[:, :], wg2r, rh[:, :],
                                 start=True, stop=True)
                expl = gb.tile([128, T], BF16, tag="expl",
                               name=f"expl_{rep}_{ti}")
                nc.scalar.activation(expl[:, :], pg2[:, :], AF.Exp, bias=bg2r)
                return expl

            def gate_c(expl, ti, rep):
                """exp-sum matmul (strip ti -> output partition 0),
                reciprocal, then the broadcast bounce: raw exp rows + the
                1/sum row go to DRAM and come back replicated to all 128
                partitions as gall[128, E+1, T]."""
                qs = quad.tile([128, T], F32, tag="q", name=f"qs_{rep}_{ti}")
                nc.tensor.matmul(qs[0:1, :], ond_ap(ti),
                                 expl[32 * ti:32 * ti + 8, :],
                                 start=True, stop=True,
                                 tile_position=(32 * ti, 0))
                invr = sb.tile([1, T], BF16, tag="invr",
                               name=f"invr_{rep}_{ti}")
                nc.vector.reciprocal(invr[0:1, :], qs[0:1, :])
                # DRAM tensors are not dependency-tracked by the tile
                # framework: chain every gd access (sync=true) so the
                # replicated reads follow the row writes (RAW) and the next
                # rep's row writes follow this rep's reads (WAR)
                key = f"gd{ti}"

                def chain(inst):
                    tc.chain_iter_dep(key, inst.ins)

                chain(nc.sync.dma_start(gd_d[ti][0:E, :], expl[0:E, :]))
                chain(nc.sync.dma_start(gd_d[ti][E:E + 1, :], invr[0:1, :]))
                gall = ga.tile([128, E + 1, T], BF16, tag="gall",
                               name=f"gall_{rep}_{ti}")
                src = gd_d[ti].ap().unsqueeze(0)
                if rep == 0:
                    # rep 0 contends with the weight stream on the serial
                    # DMA resource: two halves let experts 0..3 unblock
                    # ~1.6us earlier than a monolithic replicated read
                    chain(nc.sync.dma_start(
                        gall[:, 0:4, :],
                        src[:, 0:4, :].broadcast_to([128, 4, T])))
                    chain(nc.sync.dma_start(
                        gall[:, 4:E + 1, :],
                        src[:, 4:E + 1, :].broadcast_to([128, E + 1 - 4, T])))
                else:
                    chain(nc.sync.dma_start(
                        gall[:, :, :], src.broadcast_to([128, E + 1, T])))
                return gall

            def experts_compute(xt, ti, rep, expl, gall):
                """Layer-2 emission lags layer-1 by 1.5 experts (l2(e)
                issues between l1(e+2,mc0) and l1(e+2,mc1)) so each
                expert's relu->gate-multiply chain (~1.4us) hides under
                ~1.7us of later layer-1 passes; b2 init at e==1 gives the
                previous tile's output evacuations time to free the pout
                banks."""
                tok = slice(ti * T, (ti + 1) * T)
                py = None
                hss = [None] * E

                def emit_l2(e):
                    # kc outer: the kc=1 operand's relu finishes last, so
                    # both kc=0 passes run first and buy it ~426ns
                    for kc in range(KC):
                        for mc in range(KC):
                            nc.tensor.matmul(
                                py[mc][:, :],
                                w2[:, e, kc, mc * 128:(mc + 1) * 128],
                                hss[e][:, kc, :],
                                start=False,
                                stop=(e == E - 1 and kc == KC - 1))

                for e in range(E):
                    pt = gall[:, e, :]
                    hs = hb.tile([128, KC, T], BF16, tag="hs",
                                 name=f"hs_{rep}_{ti}_{e}")
                    hss[e] = hs
                    for mc in range(KC):
                        ph = phid.tile([128, T], F32, tag="ph",
                                       name=f"ph_{rep}_{ti}_{e}_{mc}")
                        for kc in range(KC):
                            nc.tensor.matmul(
                                ph[:, :], w1[:, e, kc, mc * 128:(mc + 1) * 128],
                                xt[:, kc, tok],
                                start=(kc == 0), stop=(kc == KC - 1))
                        nc.scalar.activation(hs[:, mc, :], ph[:, :], AF.Relu,
                                             bias=b1bias(e, mc))
                        nc.vector.tensor_mul(hs[:, mc, :], hs[:, mc, :],
                                             pt[:, :])
                        if mc == 0 and e >= 2:
                            emit_l2(e - 2)
                    if e == 1:
                        py = [pout.tile([128, T], F32, tag="py",
                                        name=f"py{mc}_{rep}_{ti}")
                              for mc in range(KC)]
                        for mc in range(KC):
                            nc.tensor.matmul(py[mc][:, :], b2blk(mc),
                                             expl[32 * mc:32 * mc + 8, :],
                                             start=True, stop=False,
                                             tile_position=(32 * mc, 0))
                emit_l2(E - 2)
                emit_l2(E - 1)
                return py

            ydst = yT_d.ap().rearrange("(mc p) t -> p mc t", p=128)

            def finalize(ti, rep, py, gall):
                tok = slice(ti * T, (ti + 1) * T)
                ot = ob.tile([128, KC, T], BF16, tag="ot",
                             name=f"ot_{rep}_{ti}")
                for mc in range(KC):
                    nc.vector.tensor_mul(ot[:, mc, :], py[mc][:, :],
                                         gall[:, E, :])
                nc.gpsimd.dma_start(ydst[:, :, tok], ot[:, :, :])

            load_weights(range(E))

            for rep in range(repeat):
                xt = xp.tile([128, KC, TPC], BF16, tag="xt", name=f"xt{rep}")
                # tile 0 alone (fast availability for gate t0), rest as one
                # transfer: fewer DMA instructions per rep
                nc.sync.dma_start(xt[:, :, 0:T], x_d[:, :, 0:T])
                nc.sync.dma_start(xt[:, :, T:TPC], x_d[:, :, T:TPC])

                # software-pipelined gate: A=pg1/relu, B=pg2/exp,
                # C=sum/recip/bounce; stage k of tile ti issues while
                # stage k+1 of tile ti-1 is still in flight. Weight loads
                # weave between the per-tile bounce DMAs so the serial DMA
                # resource serves everything in compute-need order.
                rhs = [None] * NT
                expls = [None] * NT
                galls = [None] * NT
                rhs[0] = gate_a(xt, 0, rep)
                rhs[1] = gate_a(xt, 1, rep)
                expls[0] = gate_b(rhs[0], 0, rep)
                rhs[2] = gate_a(xt, 2, rep)
                expls[1] = gate_b(rhs[1], 1, rep)
                galls[0] = gate_c(expls[0], 0, rep)
                rhs[3] = gate_a(xt, 3, rep)
                expls[2] = gate_b(rhs[2], 2, rep)
                galls[1] = gate_c(expls[1], 1, rep)
                expls[3] = gate_b(rhs[3], 3, rep)
                galls[2] = gate_c(expls[2], 2, rep)
                galls[3] = gate_c(expls[3], 3, rep)

                for ti in range(NT):
                    py = experts_compute(xt, ti, rep, expls[ti], galls[ti])
                    finalize(ti, rep, py, galls[ti])

    _split_multi_waits(nc)
    return nc


_NC_CACHE = None


def _get_nc():
    global _NC_CACHE
    if _NC_CACHE is None:
        _NC_CACHE = build_nc()
    return _NC_CACHE


def make_in_maps(x, Wg1, bg1, Wg2, bg2, W1, b1, W2, b2):
    bf = ml_dtypes.bfloat16
    x = np.ascontiguousarray(np.asarray(x, dtype=np.float32))
    Wg1 = np.asarray(Wg1, np.float32)
    bg1 = np.asarray(bg1, np.float32)
    Wg2 = np.asarray(Wg2, np.float32)
    bg2 = np.asarray(bg2, np.float32)
    W1 = np.asarray(W1, np.float32)
    b1 = np.asarray(b1, np.float32)
    W2 = np.asarray(W2, np.float32)
    b2 = np.asarray(b2, np.float32)

    gcb = np.zeros((128, GB_W), np.float32)
    gcf = np.zeros((128, GF_W), np.float32)
    # Wg1 [D, H] -> [p, kc*H + h]
    gcb[:, GB_WG1:GB_WG1 + KC * H] = (
        Wg1.reshape(KC, 128, H).transpose(1, 0, 2).reshape(128, KC * H))
    # Wg2 replicated: wg2r[h, 32s+k] = Wg2[h, k]; bg2 likewise per strip
    for s in range(4):
        gcb[:, GB_WG2 + 32 * s:GB_WG2 + 32 * s + 8] = Wg2
        gcf[32 * s:32 * s + 8, GF_BG2] = bg2
    gcf[:, GF_BG1] = bg1
    for j in range(4):
        gcb[32 * j:32 * j + 8, GB_OND] = 1.0
    gcb[0, GB_ON1:GB_ON1 + 128] = 1.0
    # b2 blocks: strip mc holds b2[:, mc*128:(mc+1)*128]
    for mc in range(KC):
        gcb[32 * mc:32 * mc + 8,
            GB_B2 + 128 * mc:GB_B2 + 128 * (mc + 1)] = b2[:, mc * 128:(mc + 1) * 128]
    # b1 as [p, e*KC + kc]
    gcf[:, GF_B1:GF_B1 + E * KC] = (
        b1.reshape(E, KC, 128).transpose(2, 0, 1).reshape(128, E * KC))

    # SBUF layouts, host-packed:
    #   x:  [N, D] -> xT [D=(kc p), N] -> [p, kc, n]
    xs = np.ascontiguousarray(
        x.T.reshape(KC, 128, N).transpose(1, 0, 2).astype(bf))
    #   W:  [E, D=(kc p), D] -> [p, e, kc, d]
    w1s = np.ascontiguousarray(
        W1.reshape(E, KC, 128, D).transpose(2, 0, 1, 3).astype(bf))
    w2s = np.ascontiguousarray(
        W2.reshape(E, KC, 128, D).transpose(2, 0, 1, 3).astype(bf))

    shared = {
        "gb": np.ascontiguousarray(gcb.astype(bf)),
        "gf": np.ascontiguousarray(gcf),
        "W1": w1s,
        "W2": w2s,
    }
    return [
        {"xs": np.ascontiguousarray(xs[:, :, c * TPC:(c + 1) * TPC]), **shared}
        for c in range(NCORES)
    ]


def gather_output(results):
    out = np.empty((N, D), np.float32)
    for c in range(NCORES):
        out[c * TPC:(c + 1) * TPC, :] = (
            np.asarray(results[c]["yT"]).astype(np.float32).T)
    return out


def kernel(x, Wg1, bg1, Wg2, bg2, W1, b1, W2, b2):
    nc = _get_nc()
    in_maps = make_in_maps(x, Wg1, bg1, Wg2, bg2, W1, b1, W2, b2)
    r = run_bass_kernel_spmd(nc, in_maps, list(range(NCORES)))
    return gather_output(r.results)
